# revision 12
# baseline (speedup 1.0000x reference)
"""Trainium2 Bass kernel for nn_Attention_70557722739202.

Standard MHA block: qkv = x @ Wqkv.T + bqkv; attn = softmax(q k^T / 8);
out = (attn v) @ Wproj.T + bproj, with B=4, N=2048, C=768, H=12, hd=64
(ratio == 1 so the slimmable slicing is identity).

Sharding (8 cores): batch x head-group.  Core c handles batch c//2 and
heads [6*(c%2), 6*(c%2)+6).  Wqkv rows / Wproj cols are sharded by head;
each core emits a partial projection output [2048, 768] and the host sums
the two partials per batch (+ bproj).

Per-core dataflow (PE matmuls in float32r = tf32 rate; U-path in bf16):
  - x.T, W slices DMA'd with input-channel on partitions.
  - q.T/k.T computed channel-major ([128, 2048] tiles, 2 heads per tile);
    v computed seq-major with a fused ones column for softmax row-sums.
    qkv biases are folded in as K=1 rank-1 matmuls.
  - S.T = k q^T per head via row-packed K=64 matmuls (2 heads concurrent
    in the PE array).  exp(S/8) runs on ScalarE straight out of PSUM with
    the 1/8 scale folded into the activation's free affine; no max
    subtraction (scores are O(1) by construction).
  - U.T = [v | 1]^T expS.T accumulated over key chunks -> rows 0..63 are
    the unnormalized attention output, row 64 the softmax denominator.
  - normalize: DVE reciprocal_approx_fast + GpSimd partition_broadcast +
    DVE multiply.
  - projection: attnT chunks (stationary) x Wproj.T slices, PSUM K-accum.

Scheduling: the attention inner loop is ScalarE-bound (exp), which leaves
the PE array under-occupied and triggers HAM re-throttling to half clock.
To keep the PE warm, the QKV matmuls of pair t+1 and the first half of the
projection are emitted as filler inside the attention loops of pair t /
pair 2.
"""

import os
import sys

for _p in ("/opt/trn_rl_repo",):
    if os.path.isdir(_p) and _p not in sys.path:
        sys.path.insert(0, _p)

import numpy as np

import concourse.bacc as bacc
import concourse.mybir as mybir
import concourse.tile as tile
from concourse.bass_utils import run_bass_kernel_spmd

DIM = 768
NHEADS = 12
B, N = 4, 2048
HD = 64          # head dim
NCORES = 8
HPC = 6          # heads per core
PAIRS = 3        # head pairs per core
GPB = 2          # head groups per batch
CH = HPC * HD    # 384 output channels per core
SCALE = (DIM // NHEADS) ** -0.5
P = 128
QT = 1024        # query tile width (PSUM: 2 banks per S tile)
NKC = N // P     # 16 key chunks
KC = DIM // P    # 6 input-channel chunks
F32 = mybir.dt.float32
F32R = mybir.dt.float32r
BF16 = mybir.dt.bfloat16
EXP = mybir.ActivationFunctionType.Exp

_PROGRAM = None


def _emit(tc, xT_d, wqkT_d, wvT_d, bqk_d, bv_d, wpT_d, y_d):
    nc = tc.nc

    from contextlib import ExitStack

    with ExitStack() as ctx:
        const = ctx.enter_context(tc.tile_pool(name="const", bufs=1))
        qkpool = ctx.enter_context(tc.tile_pool(name="qkpool", bufs=4))
        atpool = ctx.enter_context(tc.tile_pool(name="atpool", bufs=3))
        epool = ctx.enter_context(tc.tile_pool(name="epool", bufs=2))
        rpool = ctx.enter_context(tc.tile_pool(name="rpool", bufs=2))
        rbpool = ctx.enter_context(tc.tile_pool(name="rbpool", bufs=1))
        tmppool = ctx.enter_context(tc.tile_pool(name="tmppool", bufs=1))
        uspool = ctx.enter_context(tc.tile_pool(name="uspool", bufs=2))
        ypool = ctx.enter_context(tc.tile_pool(name="ypool", bufs=1))
        pspool = ctx.enter_context(tc.tile_pool(name="pspool", bufs=2, space="PSUM"))
        upool = ctx.enter_context(tc.tile_pool(name="upool", bufs=2, space="PSUM"))

        # ---- resident inputs -------------------------------------------------
        xt = const.tile([P, KC, N], F32R)       # x.T   (in-ch on partitions)
        wqk = const.tile([P, KC, 2 * CH], F32R)  # Wqk.T (in-ch on partitions)
        wv = const.tile([P, KC, CH], F32R)       # Wv.T
        wp = const.tile([P, PAIRS, DIM], F32R)   # Wproj.T slice (ch on part)
        bqk_sb = const.tile([1, 2 * CH], F32R)
        bv_sb = const.tile([1, CH], F32R)
        ones = const.tile([1, 512], F32R)
        v4 = const.tile([P, NKC, HPC * (HD + 1)], F32R)  # v + ones column

        for k in range(KC):
            nc.sync.dma_start(xt[:, k, :], xT_d[k * P:(k + 1) * P, :])
            nc.sync.dma_start(wqk[:, k, :], wqkT_d[k * P:(k + 1) * P, :])
            nc.sync.dma_start(wv[:, k, :], wvT_d[k * P:(k + 1) * P, :])
        for t in range(PAIRS):
            nc.sync.dma_start(wp[:, t, :], wpT_d[t * P:(t + 1) * P, :])
        nc.sync.dma_start(bqk_sb[:], bqk_d[:])
        nc.sync.dma_start(bv_sb[:], bv_d[:])
        # memset can't encode float32r — write through a float32 view
        nc.vector.memset(ones.bitcast(F32), 1.0)
        # Dense memset to 1.0; the v drains below only overwrite columns
        # 0..63 of each 65-wide head block, leaving column 64 == 1.0 (the
        # fused softmax-rowsum column).
        nc.vector.memset(v4.bitcast(F32), 1.0)
        v4r = v4.rearrange("p n (h c) -> p n h c", c=HD + 1)

        qk_tiles = {}   # t -> (qt, kt)
        at_tiles = []

        def emit_qkv_pair_part(t, part, nt):
            """One quarter of pair t's q.T/k.T: part in {q,k}, nt in {0,1}
            (1024-wide column group).  14 matmuls + one drain."""
            if t not in qk_tiles:
                qt_ = qkpool.tile([P, N], F32R, tag="qk", name=f"qt{t}")
                kt_ = qkpool.tile([P, N], F32R, tag="qk", name=f"kt{t}")
                qk_tiles[t] = (qt_, kt_)
            qt_, kt_ = qk_tiles[t]
            colofs = t * P if part == "q" else CH + t * P
            dst = qt_ if part == "q" else kt_
            ps = pspool.tile([P, QT], F32, tag="s", name="qkps")
            for n in range(2):
                nsl = slice(n * 512, (n + 1) * 512)
                xsl = slice(nt * QT + n * 512, nt * QT + (n + 1) * 512)
                for k in range(KC):
                    nc.tensor.matmul(
                        ps[:, nsl],
                        lhsT=wqk[:, k, colofs:colofs + P],
                        rhs=xt[:, k, xsl],
                        start=(k == 0), stop=False,
                    )
                nc.tensor.matmul(
                    ps[:, nsl],
                    lhsT=bqk_sb[:, colofs:colofs + P],
                    rhs=ones[:, 0:512],
                    start=False, stop=True,
                )
            nc.vector.tensor_copy(dst[:, nt * QT:(nt + 1) * QT], ps[:])

        def emit_v(s):
            """v for all 6 heads for sequence chunk s (with fused bias)."""
            vps = pspool.tile([P, CH], F32, tag="s", name="vps")
            for k in range(KC):
                nc.tensor.matmul(
                    vps[:],
                    lhsT=xt[:, k, s * P:(s + 1) * P],
                    rhs=wv[:, k, :],
                    start=(k == 0), stop=False,
                )
            nc.tensor.matmul(
                vps[:], lhsT=ones[:, 0:P], rhs=bv_sb[:],
                start=False, stop=True,
            )
            nc.vector.tensor_copy(
                v4r[:, s, :, 0:HD],
                vps.rearrange("p (h c) -> p h c", c=HD),
            )

        def emit_proj_mtile(s):
            """Projection for sequence chunk s: y[s*128:(s+1)*128, :]."""
            ysb = ypool.tile([P, DIM], F32, tag="y", name="ysb")
            for nh in range(2):
                pps = pspool.tile([P, DIM // 2], F32, tag="s", name="pps")
                for t in range(PAIRS):
                    nc.tensor.matmul(
                        pps[:],
                        lhsT=at_tiles[t][:, s * P:(s + 1) * P],
                        rhs=wp[:, t, nh * (DIM // 2):(nh + 1) * (DIM // 2)],
                        start=(t == 0), stop=(t == PAIRS - 1),
                    )
                nc.vector.tensor_copy(
                    ysb[:, nh * (DIM // 2):(nh + 1) * (DIM // 2)], pps[:]
                )
            nc.sync.dma_start(y_d[s * P:(s + 1) * P, :], ysb[:])

        # ---- pair 0 QKV + v (dense PE warm-up phase) ------------------------
        for part in ("q", "k"):
            for nt in range(2):
                emit_qkv_pair_part(0, part, nt)
        for s in range(NKC):
            emit_v(s)

        # ---- attention per pair, with PE filler -----------------------------
        for t in range(PAIRS):
            qt_, kt_ = qk_tiles[t]
            at = atpool.tile([P, N], F32R, tag="at", name=f"at{t}")
            at_tiles.append(at)
            # filler schedule: (j, i) -> thunk emitted after that chunk
            filler = {}
            if t < PAIRS - 1:
                parts = [("q", 0), ("q", 1), ("k", 0), ("k", 1)]
                for (prt, nt), (j_, i_) in zip(
                    parts, ((0, 3), (0, 10), (1, 3), (1, 10))
                ):
                    filler[(j_, i_)] = (
                        lambda prt=prt, nt=nt: emit_qkv_pair_part(t + 1, prt, nt)
                    )
            else:
                for s_ in range(8):
                    filler[(1, 2 * s_)] = lambda s_=s_: emit_proj_mtile(s_)
            for j in range(N // QT):
                ua = upool.tile([HD + 1, QT], F32, tag="u", name="ua")
                ub = upool.tile([HD + 1, QT], F32, tag="u", name="ub")
                for i in range(NKC):
                    sa = pspool.tile([P, QT], F32, tag="s", name="sa")
                    sb = pspool.tile([P, QT], F32, tag="s", name="sb")
                    for n in range(QT // 512):
                        qsl = slice(j * QT + n * 512, j * QT + (n + 1) * 512)
                        # Keep the two K=64 row-group matmuls (heads A at array
                        # rows 0-63, B at 64-127) adjacent in the PE stream so
                        # they execute concurrently; without this the scheduler
                        # interleaves full-array U matmuls and serializes them.
                        with tc.tile_critical():
                            nc.tensor.matmul(
                                sa[:, n * 512:(n + 1) * 512],
                                lhsT=kt_[0:HD, i * P:(i + 1) * P],
                                rhs=qt_[0:HD, qsl],
                                start=True, stop=True,
                            )
                            nc.tensor.matmul(
                                sb[:, n * 512:(n + 1) * 512],
                                lhsT=kt_[HD:P, i * P:(i + 1) * P],
                                rhs=qt_[HD:P, qsl],
                                start=True, stop=True,
                            )
                    ea = epool.tile([P, QT], F32R, tag="e", name="ea")
                    nc.scalar.activation(ea[:], sa[:], EXP, scale=SCALE)
                    eb = epool.tile([P, QT], F32R, tag="e", name="eb")
                    nc.scalar.activation(eb[:], sb[:], EXP, scale=SCALE)
                    for n in range(QT // 512):
                        nsl = slice(n * 512, (n + 1) * 512)
                        nc.tensor.matmul(
                            ua[:, nsl], lhsT=v4r[:, i, 2 * t, :], rhs=ea[:, nsl],
                            start=(i == 0), stop=(i == NKC - 1),
                        )
                        nc.tensor.matmul(
                            ub[:, nsl], lhsT=v4r[:, i, 2 * t + 1, :], rhs=eb[:, nsl],
                            start=(i == 0), stop=(i == NKC - 1),
                        )
                    if (j, i) in filler:
                        filler[(j, i)]()
                # Drain U psum to SBUF right away so the PSUM slots recycle
                # fast (the normalize chain below runs off the critical path).
                usa = uspool.tile([HD + 1, QT], F32, tag="us", name="usa")
                nc.vector.tensor_copy(usa[:], ua[:])
                usb = uspool.tile([HD + 1, QT], F32, tag="us", name="usb")
                nc.vector.tensor_copy(usb[:], ub[:])
                # normalize: out = U / rowsum  (rowsum in partition HD).
                # reciprocal_approx_fast (custom DVE op) corrupts data on HW
                # when its input sits at a non-zero base partition, so DMA the
                # rowsum row down to partition 0 first (engines can't shift
                # partitions; DMA can).
                jsl = slice(j * QT, (j + 1) * QT)
                rsa = rpool.tile([1, QT], F32, tag="rs", name="rsa", bufs=1)
                nc.sync.dma_start(rsa[:], usa[HD:HD + 1, :])
                ra = rpool.tile([1, QT], F32, tag="r", name="ra", bufs=1)
                nc.vector.reciprocal_approx_fast(ra[:], rsa[:])
                rba = rbpool.tile([HD, QT], F32, tag="rb", name="rba")
                nc.gpsimd.partition_broadcast(rba[:], ra[:])
                nc.vector.tensor_mul(at[0:HD, jsl], usa[0:HD, :], rba[:])

                rsb = rpool.tile([1, QT], F32, tag="rs", name="rsb", bufs=1)
                nc.sync.dma_start(rsb[:], usb[HD:HD + 1, :])
                rb_ = rpool.tile([1, QT], F32, tag="r", name="rb_", bufs=1)
                nc.vector.reciprocal_approx_fast(rb_[:], rsb[:])
                rbb = rbpool.tile([HD, QT], F32, tag="rb", name="rbb")
                nc.gpsimd.partition_broadcast(rbb[:], rb_[:])
                tmpb = tmppool.tile([HD, QT], F32R, tag="tmpb", name="tmpb")
                nc.vector.tensor_mul(tmpb[:], usb[0:HD, :], rbb[:])
                # move head-B channels to partitions 64..127 (engines cannot
                # shift partitions; DMA can)
                nc.sync.dma_start(at[HD:P, jsl], tmpb[:])

        # ---- remaining projection ------------------------------------------
        for s in range(8, NKC):
            emit_proj_mtile(s)


def build_program():
    nc = bacc.Bacc(
        "TRN2", target_bir_lowering=False, debug=False, num_devices=NCORES
    )
    xT_d = nc.dram_tensor("xT", [DIM, N], F32R, kind="ExternalInput").ap()
    wqkT_d = nc.dram_tensor("wqkT", [DIM, 2 * CH], F32R, kind="ExternalInput").ap()
    wvT_d = nc.dram_tensor("wvT", [DIM, CH], F32R, kind="ExternalInput").ap()
    bqk_d = nc.dram_tensor("bqk", [1, 2 * CH], F32R, kind="ExternalInput").ap()
    bv_d = nc.dram_tensor("bv", [1, CH], F32R, kind="ExternalInput").ap()
    wpT_d = nc.dram_tensor("wpT", [CH, DIM], F32R, kind="ExternalInput").ap()
    y_d = nc.dram_tensor("y", [N, DIM], F32, kind="ExternalOutput").ap()
    with tile.TileContext(nc) as tc:
        _emit(tc, xT_d, wqkT_d, wvT_d, bqk_d, bv_d, wpT_d, y_d)
    nc.compile()
    return nc


def get_program():
    global _PROGRAM
    if _PROGRAM is None:
        _PROGRAM = build_program()
    return _PROGRAM


def make_in_maps(x, Wqkv, bqkv, Wproj):
    x = np.ascontiguousarray(np.asarray(x, np.float32))
    Wqkv = np.asarray(Wqkv, np.float32)
    bqkv = np.asarray(bqkv, np.float32)
    in_maps = []
    for c in range(NCORES):
        b, g = divmod(c, GPB)
        cs = slice(g * CH, (g + 1) * CH)
        wq = Wqkv[0 * DIM:1 * DIM][cs]
        wk = Wqkv[1 * DIM:2 * DIM][cs]
        wv_ = Wqkv[2 * DIM:3 * DIM][cs]
        in_maps.append({
            "xT": np.ascontiguousarray(x[b].T),
            "wqkT": np.ascontiguousarray(np.concatenate([wq, wk], 0).T),
            "wvT": np.ascontiguousarray(wv_.T),
            "bqk": np.concatenate(
                [bqkv[0 * DIM:1 * DIM][cs], bqkv[1 * DIM:2 * DIM][cs]]
            )[None, :].copy(),
            "bv": bqkv[2 * DIM:3 * DIM][cs][None, :].copy(),
            "wpT": np.ascontiguousarray(np.asarray(Wproj, np.float32)[:, cs].T),
        })
    return in_maps


def combine_outputs(per_core_y, bproj):
    bproj = np.asarray(bproj, np.float32)
    out = np.empty((B, N, DIM), np.float32)
    for b in range(B):
        out[b] = per_core_y[GPB * b] + per_core_y[GPB * b + 1] + bproj[None, :]
    return out


def kernel(**inputs):
    ratio = int(np.asarray(inputs.get("ratio", 1)))
    assert ratio == 1, f"kernel specialized for ratio=1, got {ratio}"
    nc = get_program()
    in_maps = make_in_maps(
        inputs["x"], inputs["Wqkv"], inputs["bqkv"], inputs["Wproj"]
    )
    res = run_bass_kernel_spmd(nc, in_maps, list(range(NCORES)))
    ys = [np.asarray(res.results[c]["y"], np.float32) for c in range(NCORES)]
    return combine_outputs(ys, inputs["bproj"])


# revision 13
# speedup vs baseline: 1.9589x; 1.9589x over previous
"""Trainium2 Bass kernel for nn_Attention_70557722739202.

Standard MHA block: qkv = x @ Wqkv.T + bqkv; attn = softmax(q k^T / 8);
out = (attn v) @ Wproj.T + bproj, with B=4, N=2048, C=768, H=12, hd=64
(ratio == 1 so the slimmable slicing is identity).

Sharding (8 cores): batch x head-group.  Core c handles batch c//2 and
heads [6*(c%2), 6*(c%2)+6).  Wqkv rows / Wproj cols are sharded by head;
each core emits a partial projection output [2048, 768] and the host sums
the two partials per batch (+ bproj).

Per-core dataflow (PE matmuls in float32r = tf32 rate; U-path in bf16):
  - x.T, W slices DMA'd with input-channel on partitions.
  - q.T/k.T computed channel-major ([128, 2048] tiles, 2 heads per tile);
    v computed seq-major with a fused ones column for softmax row-sums.
    qkv biases are folded in as K=1 rank-1 matmuls.
  - S.T = k q^T per head via row-packed K=64 matmuls (2 heads concurrent
    in the PE array).  exp(S/8) runs on ScalarE straight out of PSUM with
    the 1/8 scale folded into the activation's free affine; no max
    subtraction (scores are O(1) by construction).
  - U.T = [v | 1]^T expS.T accumulated over key chunks -> rows 0..63 are
    the unnormalized attention output, row 64 the softmax denominator.
  - normalize: DVE reciprocal_approx_fast + GpSimd partition_broadcast +
    DVE multiply.
  - projection: attnT chunks (stationary) x Wproj.T slices, PSUM K-accum.

Scheduling: the attention inner loop is ScalarE-bound (exp), which leaves
the PE array under-occupied and triggers HAM re-throttling to half clock.
To keep the PE warm, the QKV matmuls of pair t+1 and the first half of the
projection are emitted as filler inside the attention loops of pair t /
pair 2.
"""

import os
import sys

for _p in ("/opt/trn_rl_repo",):
    if os.path.isdir(_p) and _p not in sys.path:
        sys.path.insert(0, _p)

import numpy as np

import concourse.bacc as bacc
import concourse.mybir as mybir
import concourse.tile as tile
from concourse.bass_utils import run_bass_kernel_spmd

DIM = 768
NHEADS = 12
B, N = 4, 2048
HD = 64          # head dim
NCORES = 8
HPC = 6          # heads per core
PAIRS = 3        # head pairs per core
GPB = 2          # head groups per batch
CH = HPC * HD    # 384 output channels per core
SCALE = (DIM // NHEADS) ** -0.5
P = 128
QT = 1024        # query tile width (PSUM: 2 banks per S tile)
NKC = N // P     # 16 key chunks
KC = DIM // P    # 6 input-channel chunks
F32 = mybir.dt.float32
F32R = mybir.dt.float32r
BF16 = mybir.dt.bfloat16
EXP = mybir.ActivationFunctionType.Exp

_PROGRAM = None


def _emit(tc, xT_d, wqkT_d, wvT_d, bqk_d, bv_d, wpT_d, y_d):
    nc = tc.nc

    from contextlib import ExitStack

    with ExitStack() as ctx:
        const = ctx.enter_context(tc.tile_pool(name="const", bufs=1))
        qkpool = ctx.enter_context(tc.tile_pool(name="qkpool", bufs=4))
        atpool = ctx.enter_context(tc.tile_pool(name="atpool", bufs=3))
        epool = ctx.enter_context(tc.tile_pool(name="epool", bufs=2))
        rpool = ctx.enter_context(tc.tile_pool(name="rpool", bufs=2))
        rbpool = ctx.enter_context(tc.tile_pool(name="rbpool", bufs=1))
        tmppool = ctx.enter_context(tc.tile_pool(name="tmppool", bufs=1))
        uspool = ctx.enter_context(tc.tile_pool(name="uspool", bufs=2))
        ypool = ctx.enter_context(tc.tile_pool(name="ypool", bufs=1))
        pspool = ctx.enter_context(tc.tile_pool(name="pspool", bufs=2, space="PSUM"))
        upool = ctx.enter_context(tc.tile_pool(name="upool", bufs=2, space="PSUM"))

        # ---- resident inputs -------------------------------------------------
        xt = const.tile([P, KC, N], F32R)       # x.T   (in-ch on partitions)
        wqk = const.tile([P, KC, 2 * CH], F32R)  # Wqk.T (in-ch on partitions)
        wv = const.tile([P, KC, CH], F32R)       # Wv.T
        wp = const.tile([P, PAIRS, DIM], F32R)   # Wproj.T slice (ch on part)
        bqk_sb = const.tile([1, 2 * CH], F32R)
        bv_sb = const.tile([1, CH], F32R)
        ones = const.tile([1, 512], F32R)
        v4 = const.tile([P, NKC, HPC * (HD + 1)], F32R)  # v + ones column

        for k in range(KC):
            nc.sync.dma_start(xt[:, k, :], xT_d[k * P:(k + 1) * P, :])
            nc.sync.dma_start(wqk[:, k, :], wqkT_d[k * P:(k + 1) * P, :])
            nc.sync.dma_start(wv[:, k, :], wvT_d[k * P:(k + 1) * P, :])
        for t in range(PAIRS):
            nc.sync.dma_start(wp[:, t, :], wpT_d[t * P:(t + 1) * P, :])
        nc.sync.dma_start(bqk_sb[:], bqk_d[:])
        nc.sync.dma_start(bv_sb[:], bv_d[:])
        # memset can't encode float32r — write through a float32 view
        nc.vector.memset(ones.bitcast(F32), 1.0)
        # Dense memset to 1.0; the v drains below only overwrite columns
        # 0..63 of each 65-wide head block, leaving column 64 == 1.0 (the
        # fused softmax-rowsum column).
        nc.vector.memset(v4.bitcast(F32), 1.0)
        v4r = v4.rearrange("p n (h c) -> p n h c", c=HD + 1)

        qk_tiles = {}   # t -> (qt, kt)
        at_tiles = []

        def emit_qkv_pair_part(t, part, nt):
            """One quarter of pair t's q.T/k.T: part in {q,k}, nt in {0,1}
            (1024-wide column group).  14 matmuls + one drain."""
            if t not in qk_tiles:
                qt_ = qkpool.tile([P, N], F32R, tag="qk", name=f"qt{t}")
                kt_ = qkpool.tile([P, N], F32R, tag="qk", name=f"kt{t}")
                qk_tiles[t] = (qt_, kt_)
            qt_, kt_ = qk_tiles[t]
            colofs = t * P if part == "q" else CH + t * P
            dst = qt_ if part == "q" else kt_
            ps = pspool.tile([P, QT], F32, tag="s", name="qkps")
            for n in range(2):
                nsl = slice(n * 512, (n + 1) * 512)
                xsl = slice(nt * QT + n * 512, nt * QT + (n + 1) * 512)
                for k in range(KC):
                    nc.tensor.matmul(
                        ps[:, nsl],
                        lhsT=wqk[:, k, colofs:colofs + P],
                        rhs=xt[:, k, xsl],
                        start=(k == 0), stop=False,
                    )
                nc.tensor.matmul(
                    ps[:, nsl],
                    lhsT=bqk_sb[:, colofs:colofs + P],
                    rhs=ones[:, 0:512],
                    start=False, stop=True,
                )
            nc.vector.tensor_copy(dst[:, nt * QT:(nt + 1) * QT], ps[:])

        def emit_v(s):
            """v for all 6 heads for sequence chunk s (with fused bias)."""
            vps = pspool.tile([P, CH], F32, tag="s", name="vps")
            for k in range(KC):
                nc.tensor.matmul(
                    vps[:],
                    lhsT=xt[:, k, s * P:(s + 1) * P],
                    rhs=wv[:, k, :],
                    start=(k == 0), stop=False,
                )
            nc.tensor.matmul(
                vps[:], lhsT=ones[:, 0:P], rhs=bv_sb[:],
                start=False, stop=True,
            )
            nc.vector.tensor_copy(
                v4r[:, s, :, 0:HD],
                vps.rearrange("p (h c) -> p h c", c=HD),
            )

        def emit_proj_mtile(s):
            """Projection for sequence chunk s: y[s*128:(s+1)*128, :]."""
            ysb = ypool.tile([P, DIM], F32, tag="y", name="ysb")
            for nh in range(2):
                pps = pspool.tile([P, DIM // 2], F32, tag="s", name="pps")
                for t in range(PAIRS):
                    nc.tensor.matmul(
                        pps[:],
                        lhsT=at_tiles[t][:, s * P:(s + 1) * P],
                        rhs=wp[:, t, nh * (DIM // 2):(nh + 1) * (DIM // 2)],
                        start=(t == 0), stop=(t == PAIRS - 1),
                    )
                nc.vector.tensor_copy(
                    ysb[:, nh * (DIM // 2):(nh + 1) * (DIM // 2)], pps[:]
                )
            nc.sync.dma_start(y_d[s * P:(s + 1) * P, :], ysb[:])

        # ---- pair 0 QKV + v (dense PE warm-up phase) ------------------------
        for part in ("q", "k"):
            for nt in range(2):
                emit_qkv_pair_part(0, part, nt)
        for s in range(NKC):
            emit_v(s)

        # ---- attention per pair, with PE filler -----------------------------
        for t in range(PAIRS):
            qt_, kt_ = qk_tiles[t]
            at = atpool.tile([P, N], F32R, tag="at", name=f"at{t}")
            at_tiles.append(at)
            # filler schedule: (j, i) -> thunk emitted after that chunk
            filler = {}
            if t < PAIRS - 1:
                parts = [("q", 0), ("q", 1), ("k", 0), ("k", 1)]
                for (prt, nt), (j_, i_) in zip(
                    parts, ((0, 3), (0, 10), (1, 3), (1, 10))
                ):
                    filler[(j_, i_)] = (
                        lambda prt=prt, nt=nt: emit_qkv_pair_part(t + 1, prt, nt)
                    )
            else:
                for s_ in range(8):
                    filler[(1, 2 * s_)] = lambda s_=s_: emit_proj_mtile(s_)
            for j in range(N // QT):
                ua = upool.tile([HD + 1, QT], F32, tag="u", name="ua")
                ub = upool.tile([HD + 1, QT], F32, tag="u", name="ub")
                # Software-pipelined emission: the U matmuls for chunk i are
                # emitted AFTER chunk i+1's S matmuls + exp, so the two K=64
                # row-group S matmuls (heads A/B at array rows 0-63/64-127)
                # keep queue priority and stay adjacent — adjacent row-group
                # pairs execute concurrently in the PE array.
                pend = None   # (ea, eb, i) waiting for its U matmuls

                def emit_u(ea, eb, i):
                    for n in range(QT // 512):
                        nsl = slice(n * 512, (n + 1) * 512)
                        nc.tensor.matmul(
                            ua[:, nsl], lhsT=v4r[:, i, 2 * t, :], rhs=ea[:, nsl],
                            start=(i == 0), stop=(i == NKC - 1),
                        )
                        nc.tensor.matmul(
                            ub[:, nsl], lhsT=v4r[:, i, 2 * t + 1, :], rhs=eb[:, nsl],
                            start=(i == 0), stop=(i == NKC - 1),
                        )

                for i in range(NKC):
                    sa = pspool.tile([P, QT], F32, tag="s", name="sa")
                    sb = pspool.tile([P, QT], F32, tag="s", name="sb")
                    for n in range(QT // 512):
                        qsl = slice(j * QT + n * 512, j * QT + (n + 1) * 512)
                        nc.tensor.matmul(
                            sa[:, n * 512:(n + 1) * 512],
                            lhsT=kt_[0:HD, i * P:(i + 1) * P],
                            rhs=qt_[0:HD, qsl],
                            start=True, stop=True,
                        )
                        nc.tensor.matmul(
                            sb[:, n * 512:(n + 1) * 512],
                            lhsT=kt_[HD:P, i * P:(i + 1) * P],
                            rhs=qt_[HD:P, qsl],
                            start=True, stop=True,
                        )
                    ea = epool.tile([P, QT], F32R, tag="e", name="ea")
                    nc.scalar.activation(ea[:], sa[:], EXP, scale=SCALE)
                    eb = epool.tile([P, QT], F32R, tag="e", name="eb")
                    nc.scalar.activation(eb[:], sb[:], EXP, scale=SCALE)
                    if pend is not None:
                        emit_u(*pend)
                    pend = (ea, eb, i)
                    if (j, i) in filler:
                        filler[(j, i)]()
                emit_u(*pend)
                # Drain U psum to SBUF right away so the PSUM slots recycle
                # fast (the normalize chain below runs off the critical path).
                usa = uspool.tile([HD + 1, QT], F32, tag="us", name="usa")
                nc.vector.tensor_copy(usa[:], ua[:])
                usb = uspool.tile([HD + 1, QT], F32, tag="us", name="usb")
                nc.vector.tensor_copy(usb[:], ub[:])
                # normalize: out = U / rowsum  (rowsum in partition HD).
                # reciprocal_approx_fast (custom DVE op) corrupts data on HW
                # when its input sits at a non-zero base partition, so DMA the
                # rowsum row down to partition 0 first (engines can't shift
                # partitions; DMA can).
                jsl = slice(j * QT, (j + 1) * QT)
                rsa = rpool.tile([1, QT], F32, tag="rs", name="rsa", bufs=1)
                nc.sync.dma_start(rsa[:], usa[HD:HD + 1, :])
                ra = rpool.tile([1, QT], F32, tag="r", name="ra", bufs=1)
                nc.vector.reciprocal_approx_fast(ra[:], rsa[:])
                rba = rbpool.tile([HD, QT], F32, tag="rb", name="rba")
                nc.gpsimd.partition_broadcast(rba[:], ra[:])
                nc.vector.tensor_mul(at[0:HD, jsl], usa[0:HD, :], rba[:])

                rsb = rpool.tile([1, QT], F32, tag="rs", name="rsb", bufs=1)
                nc.sync.dma_start(rsb[:], usb[HD:HD + 1, :])
                rb_ = rpool.tile([1, QT], F32, tag="r", name="rb_", bufs=1)
                nc.vector.reciprocal_approx_fast(rb_[:], rsb[:])
                rbb = rbpool.tile([HD, QT], F32, tag="rb", name="rbb")
                nc.gpsimd.partition_broadcast(rbb[:], rb_[:])
                tmpb = tmppool.tile([HD, QT], F32R, tag="tmpb", name="tmpb")
                nc.vector.tensor_mul(tmpb[:], usb[0:HD, :], rbb[:])
                # move head-B channels to partitions 64..127 (engines cannot
                # shift partitions; DMA can)
                nc.sync.dma_start(at[HD:P, jsl], tmpb[:])

        # ---- remaining projection ------------------------------------------
        for s in range(8, NKC):
            emit_proj_mtile(s)


def build_program():
    nc = bacc.Bacc(
        "TRN2", target_bir_lowering=False, debug=False, num_devices=NCORES
    )
    xT_d = nc.dram_tensor("xT", [DIM, N], F32R, kind="ExternalInput").ap()
    wqkT_d = nc.dram_tensor("wqkT", [DIM, 2 * CH], F32R, kind="ExternalInput").ap()
    wvT_d = nc.dram_tensor("wvT", [DIM, CH], F32R, kind="ExternalInput").ap()
    bqk_d = nc.dram_tensor("bqk", [1, 2 * CH], F32R, kind="ExternalInput").ap()
    bv_d = nc.dram_tensor("bv", [1, CH], F32R, kind="ExternalInput").ap()
    wpT_d = nc.dram_tensor("wpT", [CH, DIM], F32R, kind="ExternalInput").ap()
    y_d = nc.dram_tensor("y", [N, DIM], F32, kind="ExternalOutput").ap()
    with tile.TileContext(nc) as tc:
        _emit(tc, xT_d, wqkT_d, wvT_d, bqk_d, bv_d, wpT_d, y_d)
    nc.compile()
    return nc


def get_program():
    global _PROGRAM
    if _PROGRAM is None:
        _PROGRAM = build_program()
    return _PROGRAM


def make_in_maps(x, Wqkv, bqkv, Wproj):
    x = np.ascontiguousarray(np.asarray(x, np.float32))
    Wqkv = np.asarray(Wqkv, np.float32)
    bqkv = np.asarray(bqkv, np.float32)
    in_maps = []
    for c in range(NCORES):
        b, g = divmod(c, GPB)
        cs = slice(g * CH, (g + 1) * CH)
        wq = Wqkv[0 * DIM:1 * DIM][cs]
        wk = Wqkv[1 * DIM:2 * DIM][cs]
        wv_ = Wqkv[2 * DIM:3 * DIM][cs]
        in_maps.append({
            "xT": np.ascontiguousarray(x[b].T),
            "wqkT": np.ascontiguousarray(np.concatenate([wq, wk], 0).T),
            "wvT": np.ascontiguousarray(wv_.T),
            "bqk": np.concatenate(
                [bqkv[0 * DIM:1 * DIM][cs], bqkv[1 * DIM:2 * DIM][cs]]
            )[None, :].copy(),
            "bv": bqkv[2 * DIM:3 * DIM][cs][None, :].copy(),
            "wpT": np.ascontiguousarray(np.asarray(Wproj, np.float32)[:, cs].T),
        })
    return in_maps


def combine_outputs(per_core_y, bproj):
    bproj = np.asarray(bproj, np.float32)
    out = np.empty((B, N, DIM), np.float32)
    for b in range(B):
        out[b] = per_core_y[GPB * b] + per_core_y[GPB * b + 1] + bproj[None, :]
    return out


def kernel(**inputs):
    ratio = int(np.asarray(inputs.get("ratio", 1)))
    assert ratio == 1, f"kernel specialized for ratio=1, got {ratio}"
    nc = get_program()
    in_maps = make_in_maps(
        inputs["x"], inputs["Wqkv"], inputs["bqkv"], inputs["Wproj"]
    )
    res = run_bass_kernel_spmd(nc, in_maps, list(range(NCORES)))
    ys = [np.asarray(res.results[c]["y"], np.float32) for c in range(NCORES)]
    return combine_outputs(ys, inputs["bproj"])


# revision 15
# speedup vs baseline: 2.0476x; 1.0453x over previous
"""Trainium2 Bass kernel for nn_Attention_70557722739202.

Standard MHA block: qkv = x @ Wqkv.T + bqkv; attn = softmax(q k^T / 8);
out = (attn v) @ Wproj.T + bproj, with B=4, N=2048, C=768, H=12, hd=64
(ratio == 1 so the slimmable slicing is identity).

Sharding (8 cores): batch x head-group.  Core c handles batch c//2 and
heads [6*(c%2), 6*(c%2)+6).  Wqkv rows / Wproj cols are sharded by head;
each core emits a partial projection output [2048, 768] and the host sums
the two partials per batch (+ bproj).

Per-core dataflow (all PE matmuls in float32r = tf32 rate, fp32 PSUM):
  - x.T, W slices DMA'd with input-channel on partitions.
  - q.T/k.T computed channel-major ([128, 2048] tiles, 2 heads per tile);
    v computed seq-major with a fused ones column for softmax row-sums.
    qkv biases are folded in as K=1 rank-1 matmuls.
  - S.T = k q^T per head via row-packed K=64 matmuls (2 heads concurrent
    in the PE array).  exp(S/8) runs on ScalarE straight out of PSUM with
    the 1/8 scale folded into the activation's free affine; no max
    subtraction (scores are O(1) by construction).
  - U.T = [v | 1]^T expS.T accumulated over key chunks -> rows 0..63 are
    the unnormalized attention output, row 64 the softmax denominator.
  - normalize: DVE reciprocal_approx_fast + GpSimd partition_broadcast +
    DVE multiply.
  - projection: attnT chunks (stationary) x Wproj.T slices, PSUM K-accum.

Scheduling: the attention inner loop would leave the PE array
under-occupied (exp on ScalarE gates it), which triggers HAM re-throttling
to half clock and makes the PE the bottleneck.  Three measures keep it
warm and dense: (1) the U matmuls of chunk i are emitted after chunk
i+1's S matmuls + exp (software pipelining that also keeps the two
row-group S matmuls adjacent, so they run concurrently in the array);
(2) the QKV matmuls of pair t+1 are emitted as filler inside pair t's
attention; (3) the first half of the projection is filler inside pair
2's second query tile.  Measured on HW: 652us (naive schedule) -> 473us.
"""

import os
import sys

for _p in ("/opt/trn_rl_repo",):
    if os.path.isdir(_p) and _p not in sys.path:
        sys.path.insert(0, _p)

import numpy as np

import concourse.bacc as bacc
import concourse.mybir as mybir
import concourse.tile as tile
from concourse.bass_utils import run_bass_kernel_spmd

DIM = 768
NHEADS = 12
B, N = 4, 2048
HD = 64          # head dim
NCORES = 8
HPC = 6          # heads per core
PAIRS = 3        # head pairs per core
GPB = 2          # head groups per batch
CH = HPC * HD    # 384 output channels per core
SCALE = (DIM // NHEADS) ** -0.5
P = 128
QT = 1024        # query tile width (PSUM: 2 banks per S tile)
NKC = N // P     # 16 key chunks
KC = DIM // P    # 6 input-channel chunks
F32 = mybir.dt.float32
F32R = mybir.dt.float32r
BF16 = mybir.dt.bfloat16
EXP = mybir.ActivationFunctionType.Exp

_PROGRAM = None


def _emit(tc, xT_d, wqkT_d, wvT_d, bqk_d, bv_d, wpT_d, y_d):
    nc = tc.nc

    from contextlib import ExitStack

    with ExitStack() as ctx:
        const = ctx.enter_context(tc.tile_pool(name="const", bufs=1))
        qkpool = ctx.enter_context(tc.tile_pool(name="qkpool", bufs=4))
        atpool = ctx.enter_context(tc.tile_pool(name="atpool", bufs=3))
        epool = ctx.enter_context(tc.tile_pool(name="epool", bufs=2))
        rpool = ctx.enter_context(tc.tile_pool(name="rpool", bufs=2))
        rbpool = ctx.enter_context(tc.tile_pool(name="rbpool", bufs=1))
        tmppool = ctx.enter_context(tc.tile_pool(name="tmppool", bufs=1))
        uspool = ctx.enter_context(tc.tile_pool(name="uspool", bufs=2))
        ypool = ctx.enter_context(tc.tile_pool(name="ypool", bufs=1))
        pspool = ctx.enter_context(tc.tile_pool(name="pspool", bufs=2, space="PSUM"))
        upool = ctx.enter_context(tc.tile_pool(name="upool", bufs=2, space="PSUM"))

        # ---- resident inputs -------------------------------------------------
        xt = const.tile([P, KC, N], F32R)       # x.T   (in-ch on partitions)
        wqk = const.tile([P, KC, 2 * CH], F32R)  # Wqk.T (in-ch on partitions)
        wv = const.tile([P, KC, CH], F32R)       # Wv.T
        wp = const.tile([P, PAIRS, DIM], F32R)   # Wproj.T slice (ch on part)
        bqk_sb = const.tile([1, 2 * CH], F32R)
        bv_sb = const.tile([1, CH], F32R)
        ones = const.tile([1, 512], F32R)
        v4 = const.tile([P, NKC, HPC * (HD + 1)], F32R)  # v + ones column

        for k in range(KC):
            nc.sync.dma_start(xt[:, k, :], xT_d[k * P:(k + 1) * P, :])
            nc.sync.dma_start(wqk[:, k, :], wqkT_d[k * P:(k + 1) * P, :])
            nc.sync.dma_start(wv[:, k, :], wvT_d[k * P:(k + 1) * P, :])
        for t in range(PAIRS):
            nc.sync.dma_start(wp[:, t, :], wpT_d[t * P:(t + 1) * P, :])
        nc.sync.dma_start(bqk_sb[:], bqk_d[:])
        nc.sync.dma_start(bv_sb[:], bv_d[:])
        # memset can't encode float32r — write through a float32 view
        nc.vector.memset(ones.bitcast(F32), 1.0)
        # Dense memset to 1.0; the v drains below only overwrite columns
        # 0..63 of each 65-wide head block, leaving column 64 == 1.0 (the
        # fused softmax-rowsum column).
        nc.vector.memset(v4.bitcast(F32), 1.0)
        v4r = v4.rearrange("p n (h c) -> p n h c", c=HD + 1)

        qk_tiles = {}   # t -> (qt, kt)
        at_tiles = []

        def emit_qkv_pair_part(t, part, nt):
            """One quarter of pair t's q.T/k.T: part in {q,k}, nt in {0,1}
            (1024-wide column group).  14 matmuls + one drain."""
            if t not in qk_tiles:
                qt_ = qkpool.tile([P, N], F32R, tag="qk", name=f"qt{t}")
                kt_ = qkpool.tile([P, N], F32R, tag="qk", name=f"kt{t}")
                qk_tiles[t] = (qt_, kt_)
            qt_, kt_ = qk_tiles[t]
            colofs = t * P if part == "q" else CH + t * P
            dst = qt_ if part == "q" else kt_
            ps = pspool.tile([P, QT], F32, tag="s", name="qkps")
            for n in range(2):
                nsl = slice(n * 512, (n + 1) * 512)
                xsl = slice(nt * QT + n * 512, nt * QT + (n + 1) * 512)
                for k in range(KC):
                    nc.tensor.matmul(
                        ps[:, nsl],
                        lhsT=wqk[:, k, colofs:colofs + P],
                        rhs=xt[:, k, xsl],
                        start=(k == 0), stop=False,
                    )
                nc.tensor.matmul(
                    ps[:, nsl],
                    lhsT=bqk_sb[:, colofs:colofs + P],
                    rhs=ones[:, 0:512],
                    start=False, stop=True,
                )
            nc.vector.tensor_copy(dst[:, nt * QT:(nt + 1) * QT], ps[:])

        def emit_v(s):
            """v for all 6 heads for sequence chunk s (with fused bias)."""
            vps = pspool.tile([P, CH], F32, tag="s", name="vps")
            for k in range(KC):
                nc.tensor.matmul(
                    vps[:],
                    lhsT=xt[:, k, s * P:(s + 1) * P],
                    rhs=wv[:, k, :],
                    start=(k == 0), stop=False,
                )
            nc.tensor.matmul(
                vps[:], lhsT=ones[:, 0:P], rhs=bv_sb[:],
                start=False, stop=True,
            )
            nc.vector.tensor_copy(
                v4r[:, s, :, 0:HD],
                vps.rearrange("p (h c) -> p h c", c=HD),
            )

        def emit_proj_mtile(s):
            """Projection for sequence chunk s: y[s*128:(s+1)*128, :]."""
            ysb = ypool.tile([P, DIM], F32, tag="y", name="ysb")
            for nh in range(2):
                pps = pspool.tile([P, DIM // 2], F32, tag="s", name="pps")
                for t in range(PAIRS):
                    nc.tensor.matmul(
                        pps[:],
                        lhsT=at_tiles[t][:, s * P:(s + 1) * P],
                        rhs=wp[:, t, nh * (DIM // 2):(nh + 1) * (DIM // 2)],
                        start=(t == 0), stop=(t == PAIRS - 1),
                    )
                nc.vector.tensor_copy(
                    ysb[:, nh * (DIM // 2):(nh + 1) * (DIM // 2)], pps[:]
                )
            nc.sync.dma_start(y_d[s * P:(s + 1) * P, :], ysb[:])

        # ---- pair 0 QKV + v (dense PE warm-up phase) ------------------------
        for part in ("q", "k"):
            for nt in range(2):
                emit_qkv_pair_part(0, part, nt)
        for s in range(NKC):
            emit_v(s)

        # ---- attention per pair, with PE filler -----------------------------
        for t in range(PAIRS):
            qt_, kt_ = qk_tiles[t]
            at = atpool.tile([P, N], F32R, tag="at", name=f"at{t}")
            at_tiles.append(at)
            # filler schedule: (j, i) -> thunk emitted after that chunk
            filler = {}
            if t < PAIRS - 1:
                parts = [("q", 0), ("q", 1), ("k", 0), ("k", 1)]
                for (prt, nt), (j_, i_) in zip(
                    parts, ((0, 3), (0, 10), (1, 3), (1, 10))
                ):
                    filler[(j_, i_)] = (
                        lambda prt=prt, nt=nt: emit_qkv_pair_part(t + 1, prt, nt)
                    )
            else:
                for s_ in range(8):
                    filler[(1, 2 * s_)] = lambda s_=s_: emit_proj_mtile(s_)
            for j in range(N // QT):
                ua = upool.tile([HD + 1, QT], F32, tag="u", name="ua")
                ub = upool.tile([HD + 1, QT], F32, tag="u", name="ub")
                # Software-pipelined emission: the U matmuls for chunk i are
                # emitted AFTER chunk i+1's S matmuls + exp, so the two K=64
                # row-group S matmuls (heads A/B at array rows 0-63/64-127)
                # keep queue priority and stay adjacent — adjacent row-group
                # pairs execute concurrently in the PE array.
                pend = None   # (ea, eb, i) waiting for its U matmuls

                def emit_u(ea, eb, i):
                    for n in range(QT // 512):
                        nsl = slice(n * 512, (n + 1) * 512)
                        nc.tensor.matmul(
                            ua[:, nsl], lhsT=v4r[:, i, 2 * t, :], rhs=ea[:, nsl],
                            start=(i == 0), stop=(i == NKC - 1),
                        )
                        nc.tensor.matmul(
                            ub[:, nsl], lhsT=v4r[:, i, 2 * t + 1, :], rhs=eb[:, nsl],
                            start=(i == 0), stop=(i == NKC - 1),
                        )

                for i in range(NKC):
                    sa = pspool.tile([P, QT], F32, tag="s", name="sa")
                    sb = pspool.tile([P, QT], F32, tag="s", name="sb")
                    for n in range(QT // 512):
                        qsl = slice(j * QT + n * 512, j * QT + (n + 1) * 512)
                        nc.tensor.matmul(
                            sa[:, n * 512:(n + 1) * 512],
                            lhsT=kt_[0:HD, i * P:(i + 1) * P],
                            rhs=qt_[0:HD, qsl],
                            start=True, stop=True,
                        )
                        nc.tensor.matmul(
                            sb[:, n * 512:(n + 1) * 512],
                            lhsT=kt_[HD:P, i * P:(i + 1) * P],
                            rhs=qt_[HD:P, qsl],
                            start=True, stop=True,
                        )
                    ea = epool.tile([P, QT], F32R, tag="e", name="ea")
                    nc.scalar.activation(ea[:], sa[:], EXP, scale=SCALE)
                    eb = epool.tile([P, QT], F32R, tag="e", name="eb")
                    nc.scalar.activation(eb[:], sb[:], EXP, scale=SCALE)
                    if pend is not None:
                        emit_u(*pend)
                    pend = (ea, eb, i)
                    if (j, i) in filler:
                        filler[(j, i)]()
                emit_u(*pend)
                # Drain U psum to SBUF right away so the PSUM slots recycle
                # fast (the normalize chain below runs off the critical path).
                usa = uspool.tile([HD + 1, QT], F32, tag="us", name="usa")
                nc.vector.tensor_copy(usa[:], ua[:])
                usb = uspool.tile([HD + 1, QT], F32, tag="us", name="usb")
                nc.vector.tensor_copy(usb[:], ub[:])
                # normalize: out = U / rowsum  (rowsum in partition HD).
                # reciprocal_approx_fast (custom DVE op) corrupts data on HW
                # when its input sits at a non-zero base partition, so DMA the
                # rowsum row down to partition 0 first (engines can't shift
                # partitions; DMA can).
                jsl = slice(j * QT, (j + 1) * QT)
                rsa = rpool.tile([1, QT], F32, tag="rs", name="rsa", bufs=1)
                nc.sync.dma_start(rsa[:], usa[HD:HD + 1, :])
                ra = rpool.tile([1, QT], F32, tag="r", name="ra", bufs=1)
                nc.vector.reciprocal_approx_fast(ra[:], rsa[:])
                rba = rbpool.tile([HD, QT], F32, tag="rb", name="rba")
                nc.gpsimd.partition_broadcast(rba[:], ra[:])
                nc.vector.tensor_mul(at[0:HD, jsl], usa[0:HD, :], rba[:])

                rsb = rpool.tile([1, QT], F32, tag="rs", name="rsb", bufs=1)
                nc.sync.dma_start(rsb[:], usb[HD:HD + 1, :])
                rb_ = rpool.tile([1, QT], F32, tag="r", name="rb_", bufs=1)
                nc.vector.reciprocal_approx_fast(rb_[:], rsb[:])
                rbb = rbpool.tile([HD, QT], F32, tag="rb", name="rbb")
                nc.gpsimd.partition_broadcast(rbb[:], rb_[:])
                tmpb = tmppool.tile([HD, QT], F32R, tag="tmpb", name="tmpb")
                nc.vector.tensor_mul(tmpb[:], usb[0:HD, :], rbb[:])
                # move head-B channels to partitions 64..127 (engines cannot
                # shift partitions; DMA can)
                nc.sync.dma_start(at[HD:P, jsl], tmpb[:])

        # ---- remaining projection ------------------------------------------
        for s in range(8, NKC):
            emit_proj_mtile(s)


def build_program():
    nc = bacc.Bacc(
        "TRN2", target_bir_lowering=False, debug=False, num_devices=NCORES
    )
    xT_d = nc.dram_tensor("xT", [DIM, N], F32R, kind="ExternalInput").ap()
    wqkT_d = nc.dram_tensor("wqkT", [DIM, 2 * CH], F32R, kind="ExternalInput").ap()
    wvT_d = nc.dram_tensor("wvT", [DIM, CH], F32R, kind="ExternalInput").ap()
    bqk_d = nc.dram_tensor("bqk", [1, 2 * CH], F32R, kind="ExternalInput").ap()
    bv_d = nc.dram_tensor("bv", [1, CH], F32R, kind="ExternalInput").ap()
    wpT_d = nc.dram_tensor("wpT", [CH, DIM], F32R, kind="ExternalInput").ap()
    y_d = nc.dram_tensor("y", [N, DIM], F32, kind="ExternalOutput").ap()
    with tile.TileContext(nc) as tc:
        _emit(tc, xT_d, wqkT_d, wvT_d, bqk_d, bv_d, wpT_d, y_d)
    nc.compile()
    return nc


def get_program():
    global _PROGRAM
    if _PROGRAM is None:
        _PROGRAM = build_program()
    return _PROGRAM


def make_in_maps(x, Wqkv, bqkv, Wproj):
    x = np.ascontiguousarray(np.asarray(x, np.float32))
    Wqkv = np.asarray(Wqkv, np.float32)
    bqkv = np.asarray(bqkv, np.float32)
    in_maps = []
    for c in range(NCORES):
        b, g = divmod(c, GPB)
        cs = slice(g * CH, (g + 1) * CH)
        wq = Wqkv[0 * DIM:1 * DIM][cs]
        wk = Wqkv[1 * DIM:2 * DIM][cs]
        wv_ = Wqkv[2 * DIM:3 * DIM][cs]
        in_maps.append({
            "xT": np.ascontiguousarray(x[b].T),
            "wqkT": np.ascontiguousarray(np.concatenate([wq, wk], 0).T),
            "wvT": np.ascontiguousarray(wv_.T),
            "bqk": np.concatenate(
                [bqkv[0 * DIM:1 * DIM][cs], bqkv[1 * DIM:2 * DIM][cs]]
            )[None, :].copy(),
            "bv": bqkv[2 * DIM:3 * DIM][cs][None, :].copy(),
            "wpT": np.ascontiguousarray(np.asarray(Wproj, np.float32)[:, cs].T),
        })
    return in_maps


def combine_outputs(per_core_y, bproj):
    bproj = np.asarray(bproj, np.float32)
    out = np.empty((B, N, DIM), np.float32)
    for b in range(B):
        out[b] = per_core_y[GPB * b] + per_core_y[GPB * b + 1] + bproj[None, :]
    return out


def kernel(**inputs):
    ratio = int(np.asarray(inputs.get("ratio", 1)))
    assert ratio == 1, f"kernel specialized for ratio=1, got {ratio}"
    nc = get_program()
    in_maps = make_in_maps(
        inputs["x"], inputs["Wqkv"], inputs["bqkv"], inputs["Wproj"]
    )
    res = run_bass_kernel_spmd(nc, in_maps, list(range(NCORES)))
    ys = [np.asarray(res.results[c]["y"], np.float32) for c in range(NCORES)]
    return combine_outputs(ys, inputs["bproj"])


# revision 17
# speedup vs baseline: 2.1803x; 1.0648x over previous
"""Trainium2 Bass kernel for nn_Attention_70557722739202.

Standard MHA block: qkv = x @ Wqkv.T + bqkv; attn = softmax(q k^T / 8);
out = (attn v) @ Wproj.T + bproj, with B=4, N=2048, C=768, H=12, hd=64
(ratio == 1 so the slimmable slicing is identity).

Sharding (8 cores): batch x head-group.  Core c handles batch c//2 and
heads [6*(c%2), 6*(c%2)+6).  Wqkv rows / Wproj cols are sharded by head;
each core emits a partial projection output [2048, 768] and the host sums
the two partials per batch (+ bproj).

Per-core dataflow (all PE matmuls in float32r = tf32 rate, fp32 PSUM):
  - x.T, W slices DMA'd with input-channel on partitions.
  - q.T/k.T computed channel-major ([128, 2048] tiles, 2 heads per tile);
    v computed seq-major with a fused ones column for softmax row-sums.
    qkv biases are folded in as K=1 rank-1 matmuls.
  - S.T = k q^T per head via row-packed K=64 matmuls (2 heads concurrent
    in the PE array).  exp(S/8) runs on ScalarE straight out of PSUM with
    the 1/8 scale folded into the activation's free affine; no max
    subtraction (scores are O(1) by construction).
  - U.T = [v | 1]^T expS.T accumulated over key chunks -> rows 0..63 are
    the unnormalized attention output, row 64 the softmax denominator.
  - normalize: DVE reciprocal_approx_fast + GpSimd partition_broadcast +
    DVE multiply.
  - projection: attnT chunks (stationary) x Wproj.T slices, PSUM K-accum.

Scheduling: the attention inner loop would leave the PE array
under-occupied (exp on ScalarE gates it), which triggers HAM re-throttling
to half clock and makes the PE the bottleneck.  Three measures keep it
warm and dense: (1) the U matmuls of chunk i are emitted after chunk
i+1's S matmuls + exp (software pipelining that also keeps the two
row-group S matmuls adjacent, so they run concurrently in the array);
(2) the QKV matmuls of pair t+1 are emitted as filler inside pair t's
attention; (3) the first half of the projection is filler inside pair
2's second query tile.  Measured on HW: 652us (naive schedule) -> 473us.
"""

import os
import sys

for _p in ("/opt/trn_rl_repo",):
    if os.path.isdir(_p) and _p not in sys.path:
        sys.path.insert(0, _p)

import numpy as np

import concourse.bacc as bacc
import concourse.mybir as mybir
import concourse.tile as tile
from concourse.bass_utils import run_bass_kernel_spmd

DIM = 768
NHEADS = 12
B, N = 4, 2048
HD = 64          # head dim
NCORES = 8
HPC = 6          # heads per core
PAIRS = 3        # head pairs per core
GPB = 2          # head groups per batch
CH = HPC * HD    # 384 output channels per core
SCALE = (DIM // NHEADS) ** -0.5
P = 128
QT = 1024        # query tile width (PSUM: 2 banks per S tile)
NKC = N // P     # 16 key chunks
KC = DIM // P    # 6 input-channel chunks
F32 = mybir.dt.float32
F32R = mybir.dt.float32r
BF16 = mybir.dt.bfloat16
EXP = mybir.ActivationFunctionType.Exp

_PROGRAM = None


def _emit(tc, xT_d, wqkT_d, wvT_d, bqk_d, bv_d, wpT_d, y_d):
    nc = tc.nc

    from contextlib import ExitStack

    with ExitStack() as ctx:
        const = ctx.enter_context(tc.tile_pool(name="const", bufs=1))
        qkpool = ctx.enter_context(tc.tile_pool(name="qkpool", bufs=4))
        atpool = ctx.enter_context(tc.tile_pool(name="atpool", bufs=3))
        epool = ctx.enter_context(tc.tile_pool(name="epool", bufs=2))
        rpool = ctx.enter_context(tc.tile_pool(name="rpool", bufs=2))
        rbpool = ctx.enter_context(tc.tile_pool(name="rbpool", bufs=1))
        uspool = ctx.enter_context(tc.tile_pool(name="uspool", bufs=2))
        ypool = ctx.enter_context(tc.tile_pool(name="ypool", bufs=2))
        pspool = ctx.enter_context(tc.tile_pool(name="pspool", bufs=2, space="PSUM"))
        upool = ctx.enter_context(tc.tile_pool(name="upool", bufs=2, space="PSUM"))

        # ---- resident inputs -------------------------------------------------
        xt = const.tile([P, KC, N], F32R)       # x.T   (in-ch on partitions)
        wqk = const.tile([P, KC, 2 * CH], F32R)  # Wqk.T (in-ch on partitions)
        wv = const.tile([P, KC, CH], F32R)       # Wv.T
        wp = const.tile([P, PAIRS, DIM], F32R)   # Wproj.T slice (ch on part)
        bqk_sb = const.tile([1, 2 * CH], F32R)
        bv_sb = const.tile([1, CH], F32R)
        ones = const.tile([1, 512], F32R)
        v4 = const.tile([P, NKC, HPC * (HD + 1)], F32R)  # v + ones column

        for k in range(KC):
            nc.sync.dma_start(xt[:, k, :], xT_d[k * P:(k + 1) * P, :])
            nc.sync.dma_start(wqk[:, k, :], wqkT_d[k * P:(k + 1) * P, :])
            nc.sync.dma_start(wv[:, k, :], wvT_d[k * P:(k + 1) * P, :])
        for t in range(PAIRS):
            nc.sync.dma_start(wp[:, t, :], wpT_d[t * P:(t + 1) * P, :])
        nc.sync.dma_start(bqk_sb[:], bqk_d[:])
        nc.sync.dma_start(bv_sb[:], bv_d[:])
        # memset can't encode float32r — write through a float32 view
        nc.vector.memset(ones.bitcast(F32), 1.0)
        # Dense memset to 1.0; the v drains below only overwrite columns
        # 0..63 of each 65-wide head block, leaving column 64 == 1.0 (the
        # fused softmax-rowsum column).
        nc.vector.memset(v4.bitcast(F32), 1.0)
        v4r = v4.rearrange("p n (h c) -> p n h c", c=HD + 1)

        qk_tiles = {}   # t -> (qt, kt)
        at_tiles = []

        def emit_qkv_pair_part(t, part, nt):
            """One quarter of pair t's q.T/k.T: part in {q,k}, nt in {0,1}
            (1024-wide column group).  14 matmuls + one drain."""
            if t not in qk_tiles:
                qt_ = qkpool.tile([P, N], F32R, tag="qk", name=f"qt{t}")
                kt_ = qkpool.tile([P, N], F32R, tag="qk", name=f"kt{t}")
                qk_tiles[t] = (qt_, kt_)
            qt_, kt_ = qk_tiles[t]
            colofs = t * P if part == "q" else CH + t * P
            dst = qt_ if part == "q" else kt_
            ps = pspool.tile([P, QT], F32, tag="s", name="qkps")
            for n in range(2):
                nsl = slice(n * 512, (n + 1) * 512)
                xsl = slice(nt * QT + n * 512, nt * QT + (n + 1) * 512)
                for k in range(KC):
                    nc.tensor.matmul(
                        ps[:, nsl],
                        lhsT=wqk[:, k, colofs:colofs + P],
                        rhs=xt[:, k, xsl],
                        start=(k == 0), stop=False,
                    )
                nc.tensor.matmul(
                    ps[:, nsl],
                    lhsT=bqk_sb[:, colofs:colofs + P],
                    rhs=ones[:, 0:512],
                    start=False, stop=True,
                )
            nc.vector.tensor_copy(dst[:, nt * QT:(nt + 1) * QT], ps[:])

        def emit_v(s):
            """v for all 6 heads for sequence chunk s (with fused bias)."""
            vps = pspool.tile([P, CH], F32, tag="s", name="vps")
            for k in range(KC):
                nc.tensor.matmul(
                    vps[:],
                    lhsT=xt[:, k, s * P:(s + 1) * P],
                    rhs=wv[:, k, :],
                    start=(k == 0), stop=False,
                )
            nc.tensor.matmul(
                vps[:], lhsT=ones[:, 0:P], rhs=bv_sb[:],
                start=False, stop=True,
            )
            nc.vector.tensor_copy(
                v4r[:, s, :, 0:HD],
                vps.rearrange("p (h c) -> p h c", c=HD),
            )

        def emit_proj_mtile(s):
            """Projection for sequence chunk s: y[s*128:(s+1)*128, :]."""
            ysb = ypool.tile([P, DIM], F32, tag="y", name="ysb")
            for nh in range(2):
                pps = pspool.tile([P, DIM // 2], F32, tag="s", name="pps")
                for t in range(PAIRS):
                    nc.tensor.matmul(
                        pps[:],
                        lhsT=at_tiles[t][:, s * P:(s + 1) * P],
                        rhs=wp[:, t, nh * (DIM // 2):(nh + 1) * (DIM // 2)],
                        start=(t == 0), stop=(t == PAIRS - 1),
                    )
                nc.vector.tensor_copy(
                    ysb[:, nh * (DIM // 2):(nh + 1) * (DIM // 2)], pps[:]
                )
            nc.sync.dma_start(y_d[s * P:(s + 1) * P, :], ysb[:])

        # ---- pair 0 QKV + v (dense PE warm-up phase) ------------------------
        for part in ("q", "k"):
            for nt in range(2):
                emit_qkv_pair_part(0, part, nt)
        for s in range(NKC):
            emit_v(s)

        # ---- attention per pair, with PE filler -----------------------------
        for t in range(PAIRS):
            qt_, kt_ = qk_tiles[t]
            at = atpool.tile([P, N], F32R, tag="at", name=f"at{t}")
            at_tiles.append(at)
            # filler schedule: (j, i) -> thunk emitted after that chunk
            filler = {}
            if t < PAIRS - 1:
                parts = [("q", 0), ("q", 1), ("k", 0), ("k", 1)]
                for (prt, nt), (j_, i_) in zip(
                    parts, ((0, 3), (0, 10), (1, 3), (1, 10))
                ):
                    filler[(j_, i_)] = (
                        lambda prt=prt, nt=nt: emit_qkv_pair_part(t + 1, prt, nt)
                    )
            else:
                for s_ in range(8):
                    filler[(1, 6 + s_)] = lambda s_=s_: emit_proj_mtile(s_)
            for j in range(N // QT):
                ua = upool.tile([HD + 1, QT], F32, tag="u", name="ua")
                ub = upool.tile([HD + 1, QT], F32, tag="u", name="ub")
                # Software-pipelined emission: the U matmuls for chunk i are
                # emitted AFTER chunk i+1's S matmuls + exp, so the two K=64
                # row-group S matmuls (heads A/B at array rows 0-63/64-127)
                # keep queue priority and stay adjacent — adjacent row-group
                # pairs execute concurrently in the PE array.
                pend = None   # (ea, eb, i) waiting for its U matmuls

                def emit_u(ea, eb, i):
                    for n in range(QT // 512):
                        nsl = slice(n * 512, (n + 1) * 512)
                        nc.tensor.matmul(
                            ua[:, nsl], lhsT=v4r[:, i, 2 * t, :], rhs=ea[:, nsl],
                            start=(i == 0), stop=(i == NKC - 1),
                        )
                        nc.tensor.matmul(
                            ub[:, nsl], lhsT=v4r[:, i, 2 * t + 1, :], rhs=eb[:, nsl],
                            start=(i == 0), stop=(i == NKC - 1),
                        )

                for i in range(NKC):
                    sa = pspool.tile([P, QT], F32, tag="s", name="sa")
                    sb = pspool.tile([P, QT], F32, tag="s", name="sb")
                    for n in range(QT // 512):
                        qsl = slice(j * QT + n * 512, j * QT + (n + 1) * 512)
                        nc.tensor.matmul(
                            sa[:, n * 512:(n + 1) * 512],
                            lhsT=kt_[0:HD, i * P:(i + 1) * P],
                            rhs=qt_[0:HD, qsl],
                            start=True, stop=True,
                        )
                        nc.tensor.matmul(
                            sb[:, n * 512:(n + 1) * 512],
                            lhsT=kt_[HD:P, i * P:(i + 1) * P],
                            rhs=qt_[HD:P, qsl],
                            start=True, stop=True,
                        )
                    ea = epool.tile([P, QT], F32R, tag="e", name="ea")
                    nc.scalar.activation(ea[:], sa[:], EXP, scale=SCALE)
                    eb = epool.tile([P, QT], F32R, tag="e", name="eb")
                    nc.scalar.activation(eb[:], sb[:], EXP, scale=SCALE)
                    if pend is not None:
                        emit_u(*pend)
                    pend = (ea, eb, i)
                    if (j, i) in filler:
                        filler[(j, i)]()
                emit_u(*pend)
                # Drain U psum to SBUF right away so the PSUM slots recycle
                # fast (the normalize chain below runs off the critical path).
                usa = uspool.tile([HD + 1, QT], F32, tag="us", name="usa")
                nc.vector.tensor_copy(usa[:], ua[:])
                usb = uspool.tile([HD + 1, QT], F32, tag="us", name="usb")
                nc.vector.tensor_copy(usb[:], ub[:])
                # normalize: out = U / rowsum  (rowsum in partition HD).
                # reciprocal_approx_fast (custom DVE op) corrupts data on HW
                # when its input sits at a non-zero base partition, so DMA the
                # rowsum row down to partition 0 first (engines can't shift
                # partitions; DMA can).
                jsl = slice(j * QT, (j + 1) * QT)
                rsa = rpool.tile([1, QT], F32, tag="rs", name="rsa", bufs=1)
                nc.sync.dma_start(rsa[:], usa[HD:HD + 1, :])
                ra = rpool.tile([1, QT], F32, tag="r", name="ra", bufs=1)
                nc.vector.reciprocal_approx_fast(ra[:], rsa[:])
                rba = rbpool.tile([HD, QT], F32, tag="rb", name="rba")
                nc.gpsimd.partition_broadcast(rba[:], ra[:])
                nc.vector.tensor_mul(at[0:HD, jsl], usa[0:HD, :], rba[:])

                rsb = rpool.tile([1, QT], F32, tag="rs", name="rsb", bufs=1)
                nc.sync.dma_start(rsb[:], usb[HD:HD + 1, :])
                rb_ = rpool.tile([1, QT], F32, tag="r", name="rb_", bufs=1)
                nc.vector.reciprocal_approx_fast(rb_[:], rsb[:])
                rbb = rbpool.tile([HD, QT], F32, tag="rb", name="rbb")
                nc.gpsimd.partition_broadcast(rbb[:], rb_[:])
                # normalize head B in place (frees the SBUF a staging tile
                # would need), then DMA-shift to partitions 64..127 (engines
                # cannot shift partitions; DMA can)
                nc.vector.tensor_mul(usb[0:HD, :], usb[0:HD, :], rbb[:])
                nc.sync.dma_start(at[HD:P, jsl], usb[0:HD, :].bitcast(F32R))

        # ---- remaining projection ------------------------------------------
        for s in range(8, NKC):
            emit_proj_mtile(s)


def build_program():
    nc = bacc.Bacc(
        "TRN2", target_bir_lowering=False, debug=False, num_devices=NCORES
    )
    xT_d = nc.dram_tensor("xT", [DIM, N], F32R, kind="ExternalInput").ap()
    wqkT_d = nc.dram_tensor("wqkT", [DIM, 2 * CH], F32R, kind="ExternalInput").ap()
    wvT_d = nc.dram_tensor("wvT", [DIM, CH], F32R, kind="ExternalInput").ap()
    bqk_d = nc.dram_tensor("bqk", [1, 2 * CH], F32R, kind="ExternalInput").ap()
    bv_d = nc.dram_tensor("bv", [1, CH], F32R, kind="ExternalInput").ap()
    wpT_d = nc.dram_tensor("wpT", [CH, DIM], F32R, kind="ExternalInput").ap()
    y_d = nc.dram_tensor("y", [N, DIM], F32, kind="ExternalOutput").ap()
    with tile.TileContext(nc) as tc:
        _emit(tc, xT_d, wqkT_d, wvT_d, bqk_d, bv_d, wpT_d, y_d)
    nc.compile()
    return nc


def get_program():
    global _PROGRAM
    if _PROGRAM is None:
        _PROGRAM = build_program()
    return _PROGRAM


def make_in_maps(x, Wqkv, bqkv, Wproj):
    x = np.ascontiguousarray(np.asarray(x, np.float32))
    Wqkv = np.asarray(Wqkv, np.float32)
    bqkv = np.asarray(bqkv, np.float32)
    in_maps = []
    for c in range(NCORES):
        b, g = divmod(c, GPB)
        cs = slice(g * CH, (g + 1) * CH)
        wq = Wqkv[0 * DIM:1 * DIM][cs]
        wk = Wqkv[1 * DIM:2 * DIM][cs]
        wv_ = Wqkv[2 * DIM:3 * DIM][cs]
        in_maps.append({
            "xT": np.ascontiguousarray(x[b].T),
            "wqkT": np.ascontiguousarray(np.concatenate([wq, wk], 0).T),
            "wvT": np.ascontiguousarray(wv_.T),
            "bqk": np.concatenate(
                [bqkv[0 * DIM:1 * DIM][cs], bqkv[1 * DIM:2 * DIM][cs]]
            )[None, :].copy(),
            "bv": bqkv[2 * DIM:3 * DIM][cs][None, :].copy(),
            "wpT": np.ascontiguousarray(np.asarray(Wproj, np.float32)[:, cs].T),
        })
    return in_maps


def combine_outputs(per_core_y, bproj):
    bproj = np.asarray(bproj, np.float32)
    out = np.empty((B, N, DIM), np.float32)
    for b in range(B):
        out[b] = per_core_y[GPB * b] + per_core_y[GPB * b + 1] + bproj[None, :]
    return out


def kernel(**inputs):
    ratio = int(np.asarray(inputs.get("ratio", 1)))
    assert ratio == 1, f"kernel specialized for ratio=1, got {ratio}"
    nc = get_program()
    in_maps = make_in_maps(
        inputs["x"], inputs["Wqkv"], inputs["bqkv"], inputs["Wproj"]
    )
    res = run_bass_kernel_spmd(nc, in_maps, list(range(NCORES)))
    ys = [np.asarray(res.results[c]["y"], np.float32) for c in range(NCORES)]
    return combine_outputs(ys, inputs["bproj"])
